# revision 27
# baseline (speedup 1.0000x reference)
"""Trainium2 Bass kernel for the EdgePredictor pairwise edge-mask module.

Math (matches the jax reference):
    hi = xa @ W1[:F];  hj = xa @ W1[F:]                      # [B,C,H] each
    h  = relu(hi[:,i,:] + hj[:,j,:] + b1)                    # [B,C,C,H]
    logits = h @ W2 + b2                                     # [B,C,C,2]
    probs  = softmax((logits + g) / T), g = gumbel(key 42)
    out    = probs[..., 1]                                   # [B,C,C]

A 2-way softmax reduces exactly to a sigmoid of the logit difference:
    out = sigmoid(h @ (W2[:,1]-W2[:,0]) + (b2[1]-b2[0]) + (g1-g0))
so the device kernel computes z = w2d . relu(R + b1) + gd and sigmoids it.

Sharding: data-parallel over batch B=64 across 8 cores (8 batches/core).

Per-core dataflow (pair index p = i*64 + j, 4096 pairs per batch):
  xaT   = transpose(xa_b)                  TensorE, bf16 [F=128, C]
  S     = [hi_b ; hj_b] stacked, bf16      2 col-tiled matmuls -> [2C, H] psum
  R     = S.T-expand via Sel matmul        Sel[cstack, pair] is exact 0/1 bf16:
                                           R[h, p] = hi[i,h] + hj[j,h]
  R1    = relu(R + b1) -> bf16             ACT/DVE alternating, PSUM->SBUF
  Z     = R1-block.T @ w2d (bf16, FWL)     32 matmuls of N=1 -> [128, 32]
  out   = sigmoid(transpose(Z + gdT))      DMA to HBM
"""

import numpy as np

B, C, F, H = 64, 64, 128, 128
NCORES = 8
BPC = B // NCORES          # batches per core
PAIRS = C * C              # 4096
NCHUNK = PAIRS // 512      # 8 stage-B chunks of 512 pairs
TEMPERATURE = 1.0
GUMBEL_KEY = 42

_cache = {}


def _build_nc():
    import concourse.bacc as bacc
    import concourse.tile as tile
    from concourse import mybir

    fp32 = mybir.dt.float32
    bf16 = mybir.dt.bfloat16
    nc = bacc.Bacc("TRN2", target_bir_lowering=False, debug=False,
                   num_devices=NCORES)

    xa_s = nc.dram_tensor("xa_s", (BPC * C, F), bf16, kind="ExternalInput").ap()
    w1cat = nc.dram_tensor("w1cat", (F, 2 * H), bf16, kind="ExternalInput").ap()
    b1col = nc.dram_tensor("b1col", (H, 1), fp32, kind="ExternalInput").ap()
    w2d = nc.dram_tensor("w2d", (H, 1), bf16, kind="ExternalInput").ap()
    sel = nc.dram_tensor("sel", (2 * C, PAIRS), bf16, kind="ExternalInput").ap()
    gdt = nc.dram_tensor("gdt", (128, BPC * PAIRS // 128), fp32,
                         kind="ExternalInput").ap()
    ident = nc.dram_tensor("ident", (128, 128), fp32, kind="ExternalInput").ap()
    out_s = nc.dram_tensor("out_s", (BPC, PAIRS), fp32, kind="ExternalOutput").ap()

    Relu = mybir.ActivationFunctionType.Relu
    Sigmoid = mybir.ActivationFunctionType.Sigmoid
    TPC = PAIRS // 128     # 32 z-columns per batch

    with tile.TileContext(nc) as tc:
        with (
            tc.tile_pool(name="singles", bufs=1) as singles,
            tc.tile_pool(name="s_pool", bufs=BPC) as s_pool,
            tc.tile_pool(name="r1_pool", bufs=8) as r1_pool,
            tc.tile_pool(name="zg_pool", bufs=2) as zg_pool,
            tc.tile_pool(name="o_pool", bufs=2) as o_pool,
            tc.tile_pool(name="ps_r", bufs=6, space="PSUM") as ps_r,
            tc.tile_pool(name="ps_z", bufs=2, space="PSUM") as ps_z,
        ):
            # ---- inputs; spread across DMA queues, critical first ----
            # xa arrives pre-cast to bf16 and is transposed by the DMA
            # crossbar directly into xaT (no TensorE transpose needed)
            xaT = singles.tile([F, BPC * C], bf16)
            nc.sync.dma_start_transpose(out=xaT, in_=xa_s)
            id_sb = singles.tile([128, 128], fp32)
            nc.sync.dma_start(out=id_sb, in_=ident)
            # small weights on the vector queue
            w1_sb = singles.tile([F, 2 * H], bf16)
            nc.scalar.dma_start(out=w1_sb, in_=w1cat)
            b1_sb = singles.tile([H, 1], fp32)
            nc.scalar.dma_start(out=b1_sb, in_=b1col)
            w2_sb = singles.tile([H, 1], bf16)
            nc.scalar.dma_start(out=w2_sb, in_=w2d)
            # gumbel diffs on the scalar queue
            gd_all = singles.tile([128, BPC * TPC], fp32)
            nc.scalar.dma_start(out=gd_all, in_=gdt)

            # sel streams on the gpsimd DMA queue, chunked so stage B can
            # start once its first chunks land
            sel_sb = singles.tile([128, PAIRS], bf16)
            for k in range(0, NCHUNK, 2):
                nc.gpsimd.dma_start(out=sel_sb[:, k * 512:(k + 2) * 512],
                                    in_=sel[:, k * 512:(k + 2) * 512])

            # ---- stage A for all batches upfront (tiny matmuls) ----
            s_sbs = []
            for b in range(BPC):
                ss_full = ps_z.tile([128, H], fp32, tag="ps_z")
                ss_ps = ss_full[:, 0:H]
                xaT_b = xaT[:, b * C:(b + 1) * C]
                nc.tensor.matmul(ss_ps[0:C, :], lhsT=xaT_b,
                                 rhs=w1_sb[:, 0:H], start=True, stop=True)
                nc.tensor.matmul(ss_ps[C:2 * C, :], lhsT=xaT_b,
                                 rhs=w1_sb[:, H:2 * H], start=True, stop=True,
                                 tile_position=(0, 64))
                s_sb = s_pool.tile([128, H], bf16, tag="s_sb")
                nc.vector.tensor_copy(out=s_sb, in_=ss_ps)
                s_sbs.append(s_sb)

            # ---- main pipeline: B -> relu -> D, then per-batch postproc ----
            for b in range(BPC):
                z_ps = ps_z.tile([128, TPC], fp32, tag="ps_z")
                for kk in range(0, NCHUNK, 4):
                    # four B matmuls back-to-back share the loaded S weights
                    r_pss = []
                    r1_sbs = []
                    for k in range(kk, kk + 4):
                        r_ps = ps_r.tile([128, 512], fp32, tag="ps_r")
                        nc.tensor.matmul(r_ps, lhsT=s_sbs[b],
                                         rhs=sel_sb[:, k * 512:(k + 1) * 512],
                                         start=True, stop=True)
                        r_pss.append(r_ps)
                    for k in range(kk, kk + 4):
                        r1_sb = r1_pool.tile([128, 512], bf16, tag="r1_sb")
                        if k % 2 == 0:
                            nc.scalar.activation(out=r1_sb, in_=r_pss[k - kk],
                                                 func=Relu, bias=b1_sb,
                                                 scale=1.0)
                        else:
                            nc.vector.tensor_scalar(
                                out=r1_sb, in0=r_pss[k - kk], scalar1=b1_sb,
                                scalar2=0.0, op0=mybir.AluOpType.add,
                                op1=mybir.AluOpType.max)
                        r1_sbs.append(r1_sb)
                    for k in range(kk, kk + 4):
                        for t4 in range(4):
                            t = 4 * k + t4
                            nc.tensor.matmul(
                                z_ps[:, t:t + 1],
                                lhsT=r1_sbs[k - kk][:, t4 * 128:(t4 + 1) * 128],
                                rhs=w2_sb, start=True, stop=True)

                # ---- postproc: sigmoid(Z + gd), transposed out ----
                zg_sb = zg_pool.tile([128, TPC], fp32, tag="zg_sb")
                nc.vector.tensor_add(out=zg_sb, in0=z_ps,
                                     in1=gd_all[:, b * TPC:(b + 1) * TPC])
                zt_ps = ps_z.tile([TPC, 128], fp32, tag="ps_z")
                nc.tensor.transpose(zt_ps, zg_sb, id_sb)
                o_sb = o_pool.tile([TPC, 128], fp32, tag="o_sb")
                nc.scalar.activation(out=o_sb, in_=zt_ps, func=Sigmoid)
                nc.sync.dma_start(
                    out=out_s[b].rearrange("(t p) -> t p", p=128), in_=o_sb)

    nc.compile()
    return nc


def _host_constants(W1, b1, W2, b2):
    import ml_dtypes
    w1cat = np.concatenate([W1[:F], W1[F:]], axis=1).astype(np.float32)
    w1cat = w1cat.astype(ml_dtypes.bfloat16)
    b1col = np.ascontiguousarray(b1.astype(np.float32).reshape(H, 1))
    w2d = np.ascontiguousarray(
        ((W2[:, 1] - W2[:, 0]) / TEMPERATURE).astype(np.float32).reshape(H, 1)
    ).astype(ml_dtypes.bfloat16)
    p = np.arange(PAIRS)
    i, j = p // C, p % C
    sel = np.zeros((2 * C, PAIRS), np.float32)
    sel[i, p] += 1.0
    sel[C + j, p] += 1.0
    sel = sel.astype(ml_dtypes.bfloat16)  # 0/1 exact in bf16
    ident = np.eye(128, dtype=np.float32)
    b2d = float(b2[1] - b2[0])
    return w1cat, b1col, w2d, sel, ident, b2d


def _gumbel_diff(b2d):
    # Must match the reference bit-for-bit: same jax call on the ambient
    # default backend (the grader runs the reference in this same env).
    import jax
    import jax.numpy as jnp
    g = jax.random.gumbel(jax.random.key(GUMBEL_KEY), (B, C, C, 2), jnp.float32)
    g = np.asarray(g)
    gd = (g[..., 1] - g[..., 0] + b2d) / TEMPERATURE   # [B, C, C]
    # device layout per batch: Z[p, t] covers pair = t*128 + p
    gdt = gd.reshape(B, PAIRS // 128, 128).transpose(0, 2, 1)  # [B, 128, 32]
    # per core: [128, BPC*32] with batch-major columns
    gdt = gdt.reshape(NCORES, BPC, 128, PAIRS // 128).transpose(0, 2, 1, 3)
    return np.ascontiguousarray(gdt.astype(np.float32)).reshape(
        NCORES, 128, BPC * (PAIRS // 128))


def kernel(xa, W1, b1, W2, b2):
    from concourse.bass_utils import run_bass_kernel_spmd

    if "nc" not in _cache:
        _cache["nc"] = _build_nc()
    nc = _cache["nc"]

    w1cat, b1col, w2d, sel, ident, b2d = _host_constants(W1, b1, W2, b2)
    if _cache.get("gdt_b2d") != b2d:
        _cache["gdt"] = _gumbel_diff(b2d)
        _cache["gdt_b2d"] = b2d
    gdt = _cache["gdt"]
    import ml_dtypes
    xa_bf = np.ascontiguousarray(xa.astype(np.float32)).astype(ml_dtypes.bfloat16)

    in_maps = []
    for c in range(NCORES):
        in_maps.append({
            "xa_s": xa_bf[c * BPC:(c + 1) * BPC].reshape(BPC * C, F),
            "w1cat": w1cat, "b1col": b1col, "w2d": w2d, "sel": sel,
            "gdt": gdt[c], "ident": ident,
        })
    res = run_bass_kernel_spmd(nc, in_maps, core_ids=list(range(NCORES)))
    _cache["last_res"] = res
    out = np.concatenate([r["out_s"] for r in res.results], axis=0)
    return out.reshape(B, C, C)


# revision 28
# speedup vs baseline: 1.0158x; 1.0158x over previous
"""Trainium2 Bass kernel for the EdgePredictor pairwise edge-mask module.

Math (matches the jax reference):
    hi = xa @ W1[:F];  hj = xa @ W1[F:]                      # [B,C,H] each
    h  = relu(hi[:,i,:] + hj[:,j,:] + b1)                    # [B,C,C,H]
    logits = h @ W2 + b2                                     # [B,C,C,2]
    probs  = softmax((logits + g) / T), g = gumbel(key 42)
    out    = probs[..., 1]                                   # [B,C,C]

A 2-way softmax reduces exactly to a sigmoid of the logit difference:
    out = sigmoid(h @ (W2[:,1]-W2[:,0]) + (b2[1]-b2[0]) + (g1-g0))
so the device kernel computes z = w2d . relu(R + b1) + gd and sigmoids it.

Sharding: data-parallel over batch B=64 across 8 cores (8 batches/core).

Per-core dataflow (pair index p = i*64 + j, 4096 pairs per batch):
  xaT   = transpose(xa_b)                  TensorE, bf16 [F=128, C]
  S     = [hi_b ; hj_b] stacked, bf16      2 col-tiled matmuls -> [2C, H] psum
  R     = S.T-expand via Sel matmul        Sel[cstack, pair] is exact 0/1 bf16:
                                           R[h, p] = hi[i,h] + hj[j,h]
  R1    = relu(R + b1) -> bf16             ACT/DVE alternating, PSUM->SBUF
  Z     = R1-block.T @ w2d (bf16, FWL)     32 matmuls of N=1 -> [128, 32]
  out   = sigmoid(transpose(Z + gdT))      DMA to HBM
"""

import numpy as np

B, C, F, H = 64, 64, 128, 128
NCORES = 8
BPC = B // NCORES          # batches per core
PAIRS = C * C              # 4096
NCHUNK = PAIRS // 512      # 8 stage-B chunks of 512 pairs
TEMPERATURE = 1.0
GUMBEL_KEY = 42

_cache = {}


def _build_nc():
    import concourse.bacc as bacc
    import concourse.tile as tile
    from concourse import mybir

    fp32 = mybir.dt.float32
    bf16 = mybir.dt.bfloat16
    nc = bacc.Bacc("TRN2", target_bir_lowering=False, debug=False,
                   num_devices=NCORES)

    xa_s = nc.dram_tensor("xa_s", (BPC * C, F), bf16, kind="ExternalInput").ap()
    w1cat = nc.dram_tensor("w1cat", (F, 2 * H), bf16, kind="ExternalInput").ap()
    b1col = nc.dram_tensor("b1col", (H, 1), fp32, kind="ExternalInput").ap()
    w2d = nc.dram_tensor("w2d", (H, 1), bf16, kind="ExternalInput").ap()
    sel = nc.dram_tensor("sel", (2 * C, PAIRS), bf16, kind="ExternalInput").ap()
    gdt = nc.dram_tensor("gdt", (128, BPC * PAIRS // 128), fp32,
                         kind="ExternalInput").ap()
    ident = nc.dram_tensor("ident", (128, 128), fp32, kind="ExternalInput").ap()
    out_s = nc.dram_tensor("out_s", (BPC, PAIRS), fp32, kind="ExternalOutput").ap()

    Relu = mybir.ActivationFunctionType.Relu
    Sigmoid = mybir.ActivationFunctionType.Sigmoid
    TPC = PAIRS // 128     # 32 z-columns per batch

    with tile.TileContext(nc) as tc:
        with (
            tc.tile_pool(name="singles", bufs=1) as singles,
            tc.tile_pool(name="s_pool", bufs=BPC) as s_pool,
            tc.tile_pool(name="r1_pool", bufs=8) as r1_pool,
            tc.tile_pool(name="zg_pool", bufs=2) as zg_pool,
            tc.tile_pool(name="o_pool", bufs=2) as o_pool,
            tc.tile_pool(name="ps_r", bufs=6, space="PSUM") as ps_r,
            tc.tile_pool(name="ps_z", bufs=2, space="PSUM") as ps_z,
        ):
            # ---- inputs; spread across DMA queues, critical first ----
            # xa arrives pre-cast to bf16 and is transposed by the DMA
            # crossbar directly into xaT (no TensorE transpose needed)
            xaT = singles.tile([F, BPC * C], bf16)
            nc.sync.dma_start_transpose(out=xaT, in_=xa_s)
            # small weights on the scalar queue
            w1_sb = singles.tile([F, 2 * H], bf16)
            nc.scalar.dma_start(out=w1_sb, in_=w1cat)
            id_sb = singles.tile([128, 128], fp32)
            nc.scalar.dma_start(out=id_sb, in_=ident)
            b1_sb = singles.tile([H, 1], fp32)
            nc.scalar.dma_start(out=b1_sb, in_=b1col)
            w2_sb = singles.tile([H, 1], bf16)
            nc.scalar.dma_start(out=w2_sb, in_=w2d)
            # gumbel diffs on the scalar queue
            gd_all = singles.tile([128, BPC * TPC], fp32)
            nc.scalar.dma_start(out=gd_all, in_=gdt)

            # sel streams on the gpsimd DMA queue, chunked so stage B can
            # start once its first chunks land
            sel_sb = singles.tile([128, PAIRS], bf16)
            for k in range(0, NCHUNK, 2):
                nc.gpsimd.dma_start(out=sel_sb[:, k * 512:(k + 2) * 512],
                                    in_=sel[:, k * 512:(k + 2) * 512])

            # ---- stage A for all batches upfront (tiny matmuls) ----
            s_sbs = []
            for b in range(BPC):
                ss_full = ps_z.tile([128, H], fp32, tag="ps_z")
                ss_ps = ss_full[:, 0:H]
                xaT_b = xaT[:, b * C:(b + 1) * C]
                nc.tensor.matmul(ss_ps[0:C, :], lhsT=xaT_b,
                                 rhs=w1_sb[:, 0:H], start=True, stop=True)
                nc.tensor.matmul(ss_ps[C:2 * C, :], lhsT=xaT_b,
                                 rhs=w1_sb[:, H:2 * H], start=True, stop=True,
                                 tile_position=(0, 64))
                s_sb = s_pool.tile([128, H], bf16, tag="s_sb")
                nc.vector.tensor_copy(out=s_sb, in_=ss_ps)
                s_sbs.append(s_sb)

            # ---- main pipeline: B -> relu -> D, then per-batch postproc ----
            for b in range(BPC):
                z_ps = ps_z.tile([128, TPC], fp32, tag="ps_z")
                for kk in range(0, NCHUNK, 4):
                    # four B matmuls back-to-back share the loaded S weights
                    r_pss = []
                    r1_sbs = []
                    for k in range(kk, kk + 4):
                        r_ps = ps_r.tile([128, 512], fp32, tag="ps_r")
                        nc.tensor.matmul(r_ps, lhsT=s_sbs[b],
                                         rhs=sel_sb[:, k * 512:(k + 1) * 512],
                                         start=True, stop=True)
                        r_pss.append(r_ps)
                    for k in range(kk, kk + 4):
                        r1_sb = r1_pool.tile([128, 512], bf16, tag="r1_sb")
                        if k % 2 == 0:
                            nc.scalar.activation(out=r1_sb, in_=r_pss[k - kk],
                                                 func=Relu, bias=b1_sb,
                                                 scale=1.0)
                        else:
                            nc.vector.tensor_scalar(
                                out=r1_sb, in0=r_pss[k - kk], scalar1=b1_sb,
                                scalar2=0.0, op0=mybir.AluOpType.add,
                                op1=mybir.AluOpType.max)
                        r1_sbs.append(r1_sb)
                    for k in range(kk, kk + 4):
                        for t4 in range(4):
                            t = 4 * k + t4
                            nc.tensor.matmul(
                                z_ps[:, t:t + 1],
                                lhsT=r1_sbs[k - kk][:, t4 * 128:(t4 + 1) * 128],
                                rhs=w2_sb, start=True, stop=True)

                # ---- postproc: sigmoid(Z + gd), transposed out ----
                zg_sb = zg_pool.tile([128, TPC], fp32, tag="zg_sb")
                nc.vector.tensor_add(out=zg_sb, in0=z_ps,
                                     in1=gd_all[:, b * TPC:(b + 1) * TPC])
                zt_ps = ps_z.tile([TPC, 128], fp32, tag="ps_z")
                nc.tensor.transpose(zt_ps, zg_sb, id_sb)
                o_sb = o_pool.tile([TPC, 128], fp32, tag="o_sb")
                nc.scalar.activation(out=o_sb, in_=zt_ps, func=Sigmoid)
                nc.sync.dma_start(
                    out=out_s[b].rearrange("(t p) -> t p", p=128), in_=o_sb)

    nc.compile()
    return nc


def _host_constants(W1, b1, W2, b2):
    import ml_dtypes
    w1cat = np.concatenate([W1[:F], W1[F:]], axis=1).astype(np.float32)
    w1cat = w1cat.astype(ml_dtypes.bfloat16)
    b1col = np.ascontiguousarray(b1.astype(np.float32).reshape(H, 1))
    w2d = np.ascontiguousarray(
        ((W2[:, 1] - W2[:, 0]) / TEMPERATURE).astype(np.float32).reshape(H, 1)
    ).astype(ml_dtypes.bfloat16)
    p = np.arange(PAIRS)
    i, j = p // C, p % C
    sel = np.zeros((2 * C, PAIRS), np.float32)
    sel[i, p] += 1.0
    sel[C + j, p] += 1.0
    sel = sel.astype(ml_dtypes.bfloat16)  # 0/1 exact in bf16
    ident = np.eye(128, dtype=np.float32)
    b2d = float(b2[1] - b2[0])
    return w1cat, b1col, w2d, sel, ident, b2d


def _gumbel_diff(b2d):
    # Must match the reference bit-for-bit: same jax call on the ambient
    # default backend (the grader runs the reference in this same env).
    import jax
    import jax.numpy as jnp
    g = jax.random.gumbel(jax.random.key(GUMBEL_KEY), (B, C, C, 2), jnp.float32)
    g = np.asarray(g)
    gd = (g[..., 1] - g[..., 0] + b2d) / TEMPERATURE   # [B, C, C]
    # device layout per batch: Z[p, t] covers pair = t*128 + p
    gdt = gd.reshape(B, PAIRS // 128, 128).transpose(0, 2, 1)  # [B, 128, 32]
    # per core: [128, BPC*32] with batch-major columns
    gdt = gdt.reshape(NCORES, BPC, 128, PAIRS // 128).transpose(0, 2, 1, 3)
    return np.ascontiguousarray(gdt.astype(np.float32)).reshape(
        NCORES, 128, BPC * (PAIRS // 128))


def kernel(xa, W1, b1, W2, b2):
    from concourse.bass_utils import run_bass_kernel_spmd

    if "nc" not in _cache:
        _cache["nc"] = _build_nc()
    nc = _cache["nc"]

    w1cat, b1col, w2d, sel, ident, b2d = _host_constants(W1, b1, W2, b2)
    if _cache.get("gdt_b2d") != b2d:
        _cache["gdt"] = _gumbel_diff(b2d)
        _cache["gdt_b2d"] = b2d
    gdt = _cache["gdt"]
    import ml_dtypes
    xa_bf = np.ascontiguousarray(xa.astype(np.float32)).astype(ml_dtypes.bfloat16)

    in_maps = []
    for c in range(NCORES):
        in_maps.append({
            "xa_s": xa_bf[c * BPC:(c + 1) * BPC].reshape(BPC * C, F),
            "w1cat": w1cat, "b1col": b1col, "w2d": w2d, "sel": sel,
            "gdt": gdt[c], "ident": ident,
        })
    res = run_bass_kernel_spmd(nc, in_maps, core_ids=list(range(NCORES)))
    _cache["last_res"] = res
    out = np.concatenate([r["out_s"] for r in res.results], axis=0)
    return out.reshape(B, C, C)


# revision 29
# speedup vs baseline: 1.0544x; 1.0381x over previous
"""Trainium2 Bass kernel for the EdgePredictor pairwise edge-mask module.

Math (matches the jax reference):
    hi = xa @ W1[:F];  hj = xa @ W1[F:]                      # [B,C,H] each
    h  = relu(hi[:,i,:] + hj[:,j,:] + b1)                    # [B,C,C,H]
    logits = h @ W2 + b2                                     # [B,C,C,2]
    probs  = softmax((logits + g) / T), g = gumbel(key 42)
    out    = probs[..., 1]                                   # [B,C,C]

A 2-way softmax reduces exactly to a sigmoid of the logit difference:
    out = sigmoid(h @ (W2[:,1]-W2[:,0]) + (b2[1]-b2[0]) + (g1-g0))
so the device kernel computes z = w2d . relu(R + b1) + gd and sigmoids it.

Sharding: data-parallel over batch B=64 across 8 cores (8 batches/core).

Per-core dataflow (pair index p = i*64 + j, 4096 pairs per batch):
  xaT   = transpose(xa_b)                  TensorE, bf16 [F=128, C]
  S     = [hi_b ; hj_b] stacked, bf16      2 col-tiled matmuls -> [2C, H] psum
  R     = S.T-expand via Sel matmul        Sel[cstack, pair] is exact 0/1 bf16:
                                           R[h, p] = hi[i,h] + hj[j,h]
  R1    = relu(R + b1) -> bf16             ACT/DVE alternating, PSUM->SBUF
  Z     = R1-block.T @ w2d (bf16, FWL)     32 matmuls of N=1 -> [128, 32]
  out   = sigmoid(transpose(Z + gdT))      DMA to HBM
"""

import numpy as np

B, C, F, H = 64, 64, 128, 128
NCORES = 8
BPC = B // NCORES          # batches per core
PAIRS = C * C              # 4096
NCHUNK = PAIRS // 512      # 8 stage-B chunks of 512 pairs
TEMPERATURE = 1.0
GUMBEL_KEY = 42

_cache = {}


def _build_nc():
    import concourse.bacc as bacc
    import concourse.tile as tile
    from concourse import mybir

    fp32 = mybir.dt.float32
    bf16 = mybir.dt.bfloat16
    nc = bacc.Bacc("TRN2", target_bir_lowering=False, debug=False,
                   num_devices=NCORES)

    xa_s = nc.dram_tensor("xa_s", (BPC * C, F), fp32, kind="ExternalInput").ap()
    w1cat = nc.dram_tensor("w1cat", (F, 2 * H), bf16, kind="ExternalInput").ap()
    b1col = nc.dram_tensor("b1col", (H, 1), fp32, kind="ExternalInput").ap()
    w2d = nc.dram_tensor("w2d", (H, 1), bf16, kind="ExternalInput").ap()
    sel = nc.dram_tensor("sel", (2 * C, PAIRS), bf16, kind="ExternalInput").ap()
    gdt = nc.dram_tensor("gdt", (128, BPC * PAIRS // 128), fp32,
                         kind="ExternalInput").ap()
    ident = nc.dram_tensor("ident", (128, 128), fp32, kind="ExternalInput").ap()
    out_s = nc.dram_tensor("out_s", (BPC, PAIRS), fp32, kind="ExternalOutput").ap()

    Relu = mybir.ActivationFunctionType.Relu
    Sigmoid = mybir.ActivationFunctionType.Sigmoid
    TPC = PAIRS // 128     # 32 z-columns per batch

    with tile.TileContext(nc) as tc:
        with (
            tc.tile_pool(name="singles", bufs=1) as singles,
            tc.tile_pool(name="s_pool", bufs=BPC) as s_pool,
            tc.tile_pool(name="r1_pool", bufs=8) as r1_pool,
            tc.tile_pool(name="zg_pool", bufs=2) as zg_pool,
            tc.tile_pool(name="o_pool", bufs=2) as o_pool,
            tc.tile_pool(name="ps_misc", bufs=1, space="PSUM") as ps_misc,
            tc.tile_pool(name="ps_r", bufs=5, space="PSUM") as ps_r,
            tc.tile_pool(name="ps_z", bufs=2, space="PSUM") as ps_z,
        ):
            # ---- inputs; spread across DMA queues, critical first ----
            # xa + identity on the sync queue (gate the transposes)
            xa_sb = singles.tile([128, BPC * C // 128, F], fp32)
            nc.sync.dma_start(
                out=xa_sb,
                in_=xa_s.rearrange("(ch p) f -> p ch f", p=128))
            id_sb = singles.tile([128, 128], fp32)
            nc.sync.dma_start(out=id_sb, in_=ident)
            # small weights on the vector queue
            w1_sb = singles.tile([F, 2 * H], bf16)
            nc.scalar.dma_start(out=w1_sb, in_=w1cat)
            b1_sb = singles.tile([H, 1], fp32)
            nc.scalar.dma_start(out=b1_sb, in_=b1col)
            w2_sb = singles.tile([H, 1], bf16)
            nc.scalar.dma_start(out=w2_sb, in_=w2d)
            # gumbel diffs on the scalar queue
            gd_all = singles.tile([128, BPC * TPC], fp32)
            nc.scalar.dma_start(out=gd_all, in_=gdt)

            # sel streams on the gpsimd DMA queue, chunked so stage B can
            # start once its first chunks land
            sel_sb = singles.tile([128, PAIRS], bf16)
            for k in range(0, NCHUNK, 2):
                nc.gpsimd.dma_start(out=sel_sb[:, k * 512:(k + 2) * 512],
                                    in_=sel[:, k * 512:(k + 2) * 512])

            # ---- transpose xa: xaT[f, b*64+c], bf16 ----
            xaT = singles.tile([F, BPC * C], bf16)
            for ch in range(BPC * C // 128):
                xt_ps = ps_misc.tile([F, 128], fp32, tag="ps_misc")
                nc.tensor.transpose(xt_ps, xa_sb[:, ch, :], id_sb)
                nc.scalar.copy(out=xaT[:, ch * 128:(ch + 1) * 128], in_=xt_ps)

            # ---- stage A for all batches upfront (tiny matmuls) ----
            s_sbs = []
            for b in range(BPC):
                ss_full = ps_z.tile([128, H], fp32, tag="ps_z")
                ss_ps = ss_full[:, 0:H]
                xaT_b = xaT[:, b * C:(b + 1) * C]
                nc.tensor.matmul(ss_ps[0:C, :], lhsT=xaT_b,
                                 rhs=w1_sb[:, 0:H], start=True, stop=True)
                nc.tensor.matmul(ss_ps[C:2 * C, :], lhsT=xaT_b,
                                 rhs=w1_sb[:, H:2 * H], start=True, stop=True,
                                 tile_position=(0, 64))
                s_sb = s_pool.tile([128, H], bf16, tag="s_sb")
                nc.vector.tensor_copy(out=s_sb, in_=ss_ps)
                s_sbs.append(s_sb)

            # ---- main pipeline: B -> relu -> D, then per-batch postproc ----
            for b in range(BPC):
                z_ps = ps_z.tile([128, TPC], fp32, tag="ps_z")
                for kk in range(0, NCHUNK, 4):
                    # four B matmuls back-to-back share the loaded S weights
                    r_pss = []
                    r1_sbs = []
                    for k in range(kk, kk + 4):
                        r_ps = ps_r.tile([128, 512], fp32, tag="ps_r")
                        nc.tensor.matmul(r_ps, lhsT=s_sbs[b],
                                         rhs=sel_sb[:, k * 512:(k + 1) * 512],
                                         start=True, stop=True)
                        r_pss.append(r_ps)
                    for k in range(kk, kk + 4):
                        r1_sb = r1_pool.tile([128, 512], bf16, tag="r1_sb")
                        if k % 2 == 0:
                            nc.scalar.activation(out=r1_sb, in_=r_pss[k - kk],
                                                 func=Relu, bias=b1_sb,
                                                 scale=1.0)
                        else:
                            nc.vector.tensor_scalar(
                                out=r1_sb, in0=r_pss[k - kk], scalar1=b1_sb,
                                scalar2=0.0, op0=mybir.AluOpType.add,
                                op1=mybir.AluOpType.max)
                        r1_sbs.append(r1_sb)
                    for k in range(kk, kk + 4):
                        for t4 in range(4):
                            t = 4 * k + t4
                            nc.tensor.matmul(
                                z_ps[:, t:t + 1],
                                lhsT=r1_sbs[k - kk][:, t4 * 128:(t4 + 1) * 128],
                                rhs=w2_sb, start=True, stop=True)

                # ---- postproc: sigmoid(Z + gd), transposed out ----
                zg_sb = zg_pool.tile([128, TPC], fp32, tag="zg_sb")
                nc.vector.tensor_add(out=zg_sb, in0=z_ps,
                                     in1=gd_all[:, b * TPC:(b + 1) * TPC])
                zt_ps = ps_misc.tile([TPC, 128], fp32, tag="ps_misc")
                nc.tensor.transpose(zt_ps, zg_sb, id_sb)
                o_sb = o_pool.tile([TPC, 128], fp32, tag="o_sb")
                nc.scalar.activation(out=o_sb, in_=zt_ps, func=Sigmoid)
                nc.sync.dma_start(
                    out=out_s[b].rearrange("(t p) -> t p", p=128), in_=o_sb)

    nc.compile()
    return nc


def _host_constants(W1, b1, W2, b2):
    import ml_dtypes
    w1cat = np.concatenate([W1[:F], W1[F:]], axis=1).astype(np.float32)
    w1cat = w1cat.astype(ml_dtypes.bfloat16)
    b1col = np.ascontiguousarray(b1.astype(np.float32).reshape(H, 1))
    w2d = np.ascontiguousarray(
        ((W2[:, 1] - W2[:, 0]) / TEMPERATURE).astype(np.float32).reshape(H, 1)
    ).astype(ml_dtypes.bfloat16)
    p = np.arange(PAIRS)
    i, j = p // C, p % C
    sel = np.zeros((2 * C, PAIRS), np.float32)
    sel[i, p] += 1.0
    sel[C + j, p] += 1.0
    sel = sel.astype(ml_dtypes.bfloat16)  # 0/1 exact in bf16
    ident = np.eye(128, dtype=np.float32)
    b2d = float(b2[1] - b2[0])
    return w1cat, b1col, w2d, sel, ident, b2d


def _gumbel_diff(b2d):
    # Must match the reference bit-for-bit: same jax call on the ambient
    # default backend (the grader runs the reference in this same env).
    import jax
    import jax.numpy as jnp
    g = jax.random.gumbel(jax.random.key(GUMBEL_KEY), (B, C, C, 2), jnp.float32)
    g = np.asarray(g)
    gd = (g[..., 1] - g[..., 0] + b2d) / TEMPERATURE   # [B, C, C]
    # device layout per batch: Z[p, t] covers pair = t*128 + p
    gdt = gd.reshape(B, PAIRS // 128, 128).transpose(0, 2, 1)  # [B, 128, 32]
    # per core: [128, BPC*32] with batch-major columns
    gdt = gdt.reshape(NCORES, BPC, 128, PAIRS // 128).transpose(0, 2, 1, 3)
    return np.ascontiguousarray(gdt.astype(np.float32)).reshape(
        NCORES, 128, BPC * (PAIRS // 128))


def kernel(xa, W1, b1, W2, b2):
    from concourse.bass_utils import run_bass_kernel_spmd

    if "nc" not in _cache:
        _cache["nc"] = _build_nc()
    nc = _cache["nc"]

    w1cat, b1col, w2d, sel, ident, b2d = _host_constants(W1, b1, W2, b2)
    if _cache.get("gdt_b2d") != b2d:
        _cache["gdt"] = _gumbel_diff(b2d)
        _cache["gdt_b2d"] = b2d
    gdt = _cache["gdt"]
    xa = np.ascontiguousarray(xa.astype(np.float32))

    in_maps = []
    for c in range(NCORES):
        in_maps.append({
            "xa_s": xa[c * BPC:(c + 1) * BPC].reshape(BPC * C, F),
            "w1cat": w1cat, "b1col": b1col, "w2d": w2d, "sel": sel,
            "gdt": gdt[c], "ident": ident,
        })
    res = run_bass_kernel_spmd(nc, in_maps, core_ids=list(range(NCORES)))
    _cache["last_res"] = res
    out = np.concatenate([r["out_s"] for r in res.results], axis=0)
    return out.reshape(B, C, C)


# revision 30
# speedup vs baseline: 1.0665x; 1.0115x over previous
"""Trainium2 Bass kernel for the EdgePredictor pairwise edge-mask module.

Math (matches the jax reference):
    hi = xa @ W1[:F];  hj = xa @ W1[F:]                      # [B,C,H] each
    h  = relu(hi[:,i,:] + hj[:,j,:] + b1)                    # [B,C,C,H]
    logits = h @ W2 + b2                                     # [B,C,C,2]
    probs  = softmax((logits + g) / T), g = gumbel(key 42)
    out    = probs[..., 1]                                   # [B,C,C]

A 2-way softmax reduces exactly to a sigmoid of the logit difference:
    out = sigmoid(h @ (W2[:,1]-W2[:,0]) + (b2[1]-b2[0]) + (g1-g0))
so the device kernel computes z = w2d . relu(R + b1) + gd and sigmoids it.

Sharding: data-parallel over batch B=64 across 8 cores (8 batches/core).

Per-core dataflow (pair index p = i*64 + j, 4096 pairs per batch):
  xaT   = transpose(xa_b)                  TensorE, bf16 [F=128, C]
  S     = [hi_b ; hj_b] stacked, bf16      2 col-tiled matmuls -> [2C, H] psum
  R     = S.T-expand via Sel matmul        Sel[cstack, pair] is exact 0/1 bf16:
                                           R[h, p] = hi[i,h] + hj[j,h]
  R1    = relu(R + b1) -> bf16             ACT/DVE alternating, PSUM->SBUF
  Z     = R1-block.T @ w2d (bf16, FWL)     32 matmuls of N=1 -> [128, 32]
  out   = sigmoid(transpose(Z + gdT))      DMA to HBM
"""

import numpy as np

B, C, F, H = 64, 64, 128, 128
NCORES = 8
BPC = B // NCORES          # batches per core
PAIRS = C * C              # 4096
NCHUNK = PAIRS // 512      # 8 stage-B chunks of 512 pairs
TEMPERATURE = 1.0
GUMBEL_KEY = 42

_cache = {}


def _build_nc():
    import concourse.bacc as bacc
    import concourse.tile as tile
    from concourse import mybir

    fp32 = mybir.dt.float32
    bf16 = mybir.dt.bfloat16
    nc = bacc.Bacc("TRN2", target_bir_lowering=False, debug=False,
                   num_devices=NCORES)

    xa_s = nc.dram_tensor("xa_s", (BPC * C, F), fp32, kind="ExternalInput").ap()
    w1cat = nc.dram_tensor("w1cat", (F, 2 * H), bf16, kind="ExternalInput").ap()
    b1col = nc.dram_tensor("b1col", (H, 1), fp32, kind="ExternalInput").ap()
    w2d = nc.dram_tensor("w2d", (H, 1), bf16, kind="ExternalInput").ap()
    sel = nc.dram_tensor("sel", (2 * C, PAIRS), bf16, kind="ExternalInput").ap()
    gdt = nc.dram_tensor("gdt", (128, BPC * PAIRS // 128), fp32,
                         kind="ExternalInput").ap()
    ident = nc.dram_tensor("ident", (128, 128), fp32, kind="ExternalInput").ap()
    out_s = nc.dram_tensor("out_s", (BPC, PAIRS), fp32, kind="ExternalOutput").ap()

    Relu = mybir.ActivationFunctionType.Relu
    Sigmoid = mybir.ActivationFunctionType.Sigmoid
    TPC = PAIRS // 128     # 32 z-columns per batch

    with tile.TileContext(nc) as tc:
        with (
            tc.tile_pool(name="singles", bufs=1) as singles,
            tc.tile_pool(name="s_pool", bufs=BPC) as s_pool,
            tc.tile_pool(name="r1_pool", bufs=10) as r1_pool,
            tc.tile_pool(name="zg_pool", bufs=3) as zg_pool,
            tc.tile_pool(name="o_pool", bufs=3) as o_pool,
            tc.tile_pool(name="ps_misc", bufs=1, space="PSUM") as ps_misc,
            tc.tile_pool(name="ps_r", bufs=5, space="PSUM") as ps_r,
            tc.tile_pool(name="ps_z", bufs=2, space="PSUM") as ps_z,
        ):
            # ---- inputs; spread across DMA queues, critical first ----
            # xa + identity on the sync queue (gate the transposes)
            xa_sb = singles.tile([128, BPC * C // 128, F], fp32)
            nc.sync.dma_start(
                out=xa_sb,
                in_=xa_s.rearrange("(ch p) f -> p ch f", p=128))
            id_sb = singles.tile([128, 128], fp32)
            nc.sync.dma_start(out=id_sb, in_=ident)
            # small weights on the vector queue
            w1_sb = singles.tile([F, 2 * H], bf16)
            nc.scalar.dma_start(out=w1_sb, in_=w1cat)
            b1_sb = singles.tile([H, 1], fp32)
            nc.scalar.dma_start(out=b1_sb, in_=b1col)
            w2_sb = singles.tile([H, 1], bf16)
            nc.scalar.dma_start(out=w2_sb, in_=w2d)
            # gumbel diffs on the scalar queue
            gd_all = singles.tile([128, BPC * TPC], fp32)
            nc.scalar.dma_start(out=gd_all, in_=gdt)

            # sel streams on the gpsimd DMA queue, chunked so stage B can
            # start once its first chunks land
            sel_sb = singles.tile([128, PAIRS], bf16)
            for k in range(0, NCHUNK, 2):
                nc.gpsimd.dma_start(out=sel_sb[:, k * 512:(k + 2) * 512],
                                    in_=sel[:, k * 512:(k + 2) * 512])

            # ---- transpose xa: xaT[f, b*64+c], bf16 ----
            xaT = singles.tile([F, BPC * C], bf16)
            for ch in range(BPC * C // 128):
                xt_ps = ps_misc.tile([F, 128], fp32, tag="ps_misc")
                nc.tensor.transpose(xt_ps, xa_sb[:, ch, :], id_sb)
                nc.scalar.copy(out=xaT[:, ch * 128:(ch + 1) * 128], in_=xt_ps)

            # ---- stage A for all batches upfront (tiny matmuls) ----
            s_sbs = []
            for b in range(BPC):
                ss_full = ps_z.tile([128, H], fp32, tag="ps_z")
                ss_ps = ss_full[:, 0:H]
                xaT_b = xaT[:, b * C:(b + 1) * C]
                nc.tensor.matmul(ss_ps[0:C, :], lhsT=xaT_b,
                                 rhs=w1_sb[:, 0:H], start=True, stop=True)
                nc.tensor.matmul(ss_ps[C:2 * C, :], lhsT=xaT_b,
                                 rhs=w1_sb[:, H:2 * H], start=True, stop=True,
                                 tile_position=(0, 64))
                s_sb = s_pool.tile([128, H], bf16, tag="s_sb")
                nc.vector.tensor_copy(out=s_sb, in_=ss_ps)
                s_sbs.append(s_sb)

            # ---- main pipeline: B -> relu -> D, then per-batch postproc ----
            for b in range(BPC):
                z_ps = ps_z.tile([128, TPC], fp32, tag="ps_z")
                for kk in range(0, NCHUNK, 4):
                    # four B matmuls back-to-back share the loaded S weights
                    r_pss = []
                    r1_sbs = []
                    for k in range(kk, kk + 4):
                        r_ps = ps_r.tile([128, 512], fp32, tag="ps_r")
                        nc.tensor.matmul(r_ps, lhsT=s_sbs[b],
                                         rhs=sel_sb[:, k * 512:(k + 1) * 512],
                                         start=True, stop=True)
                        r_pss.append(r_ps)
                    for k in range(kk, kk + 4):
                        r1_sb = r1_pool.tile([128, 512], bf16, tag="r1_sb")
                        if k % 2 == 0:
                            nc.scalar.activation(out=r1_sb, in_=r_pss[k - kk],
                                                 func=Relu, bias=b1_sb,
                                                 scale=1.0)
                        else:
                            nc.vector.tensor_scalar(
                                out=r1_sb, in0=r_pss[k - kk], scalar1=b1_sb,
                                scalar2=0.0, op0=mybir.AluOpType.add,
                                op1=mybir.AluOpType.max)
                        r1_sbs.append(r1_sb)
                    for k in range(kk, kk + 4):
                        for t4 in range(4):
                            t = 4 * k + t4
                            nc.tensor.matmul(
                                z_ps[:, t:t + 1],
                                lhsT=r1_sbs[k - kk][:, t4 * 128:(t4 + 1) * 128],
                                rhs=w2_sb, start=True, stop=True)

                # ---- postproc: sigmoid(Z + gd), transposed out ----
                zg_sb = zg_pool.tile([128, TPC], fp32, tag="zg_sb")
                nc.vector.tensor_add(out=zg_sb, in0=z_ps,
                                     in1=gd_all[:, b * TPC:(b + 1) * TPC])
                zt_ps = ps_misc.tile([TPC, 128], fp32, tag="ps_misc")
                nc.tensor.transpose(zt_ps, zg_sb, id_sb)
                o_sb = o_pool.tile([TPC, 128], fp32, tag="o_sb")
                nc.scalar.activation(out=o_sb, in_=zt_ps, func=Sigmoid)
                nc.sync.dma_start(
                    out=out_s[b].rearrange("(t p) -> t p", p=128), in_=o_sb)

    nc.compile()
    return nc


def _host_constants(W1, b1, W2, b2):
    import ml_dtypes
    w1cat = np.concatenate([W1[:F], W1[F:]], axis=1).astype(np.float32)
    w1cat = w1cat.astype(ml_dtypes.bfloat16)
    b1col = np.ascontiguousarray(b1.astype(np.float32).reshape(H, 1))
    w2d = np.ascontiguousarray(
        ((W2[:, 1] - W2[:, 0]) / TEMPERATURE).astype(np.float32).reshape(H, 1)
    ).astype(ml_dtypes.bfloat16)
    p = np.arange(PAIRS)
    i, j = p // C, p % C
    sel = np.zeros((2 * C, PAIRS), np.float32)
    sel[i, p] += 1.0
    sel[C + j, p] += 1.0
    sel = sel.astype(ml_dtypes.bfloat16)  # 0/1 exact in bf16
    ident = np.eye(128, dtype=np.float32)
    b2d = float(b2[1] - b2[0])
    return w1cat, b1col, w2d, sel, ident, b2d


def _gumbel_diff(b2d):
    # Must match the reference bit-for-bit: same jax call on the ambient
    # default backend (the grader runs the reference in this same env).
    import jax
    import jax.numpy as jnp
    g = jax.random.gumbel(jax.random.key(GUMBEL_KEY), (B, C, C, 2), jnp.float32)
    g = np.asarray(g)
    gd = (g[..., 1] - g[..., 0] + b2d) / TEMPERATURE   # [B, C, C]
    # device layout per batch: Z[p, t] covers pair = t*128 + p
    gdt = gd.reshape(B, PAIRS // 128, 128).transpose(0, 2, 1)  # [B, 128, 32]
    # per core: [128, BPC*32] with batch-major columns
    gdt = gdt.reshape(NCORES, BPC, 128, PAIRS // 128).transpose(0, 2, 1, 3)
    return np.ascontiguousarray(gdt.astype(np.float32)).reshape(
        NCORES, 128, BPC * (PAIRS // 128))


def kernel(xa, W1, b1, W2, b2):
    from concourse.bass_utils import run_bass_kernel_spmd

    if "nc" not in _cache:
        _cache["nc"] = _build_nc()
    nc = _cache["nc"]

    w1cat, b1col, w2d, sel, ident, b2d = _host_constants(W1, b1, W2, b2)
    if _cache.get("gdt_b2d") != b2d:
        _cache["gdt"] = _gumbel_diff(b2d)
        _cache["gdt_b2d"] = b2d
    gdt = _cache["gdt"]
    xa = np.ascontiguousarray(xa.astype(np.float32))

    in_maps = []
    for c in range(NCORES):
        in_maps.append({
            "xa_s": xa[c * BPC:(c + 1) * BPC].reshape(BPC * C, F),
            "w1cat": w1cat, "b1col": b1col, "w2d": w2d, "sel": sel,
            "gdt": gdt[c], "ident": ident,
        })
    res = run_bass_kernel_spmd(nc, in_maps, core_ids=list(range(NCORES)))
    _cache["last_res"] = res
    out = np.concatenate([r["out_s"] for r in res.results], axis=0)
    return out.reshape(B, C, C)


# revision 32
# speedup vs baseline: 1.1264x; 1.0562x over previous
"""Trainium2 Bass kernel for the EdgePredictor pairwise edge-mask module.

Math (matches the jax reference):
    hi = xa @ W1[:F];  hj = xa @ W1[F:]                      # [B,C,H] each
    h  = relu(hi[:,i,:] + hj[:,j,:] + b1)                    # [B,C,C,H]
    logits = h @ W2 + b2                                     # [B,C,C,2]
    probs  = softmax((logits + g) / T), g = gumbel(key 42)
    out    = probs[..., 1]                                   # [B,C,C]

A 2-way softmax reduces exactly to a sigmoid of the logit difference:
    out = sigmoid(h @ (W2[:,1]-W2[:,0]) + (b2[1]-b2[0]) + (g1-g0))
so the device kernel computes z = w2d . relu(R + b1) + gd and sigmoids it.

Sharding: data-parallel over batch B=64 across 8 cores (8 batches/core).

Per-core dataflow (pair index p = i*64 + j, 4096 pairs per batch):
  xaT   = transpose(xa_b)                  TensorE, bf16 [F=128, C]
  S     = [hi_b ; hj_b] stacked, bf16      2 col-tiled matmuls -> [2C, H] psum
  R     = S.T-expand via Sel matmul        Sel[cstack, pair] is exact 0/1 bf16:
                                           R[h, p] = hi[i,h] + hj[j,h]
  R1    = relu(R + b1) -> bf16             ACT/DVE alternating, PSUM->SBUF
  Z     = R1-block.T @ w2d (bf16, FWL)     32 matmuls of N=1 -> [128, 32]
  out   = sigmoid(transpose(Z + gdT))      DMA to HBM
"""

import numpy as np

B, C, F, H = 64, 64, 128, 128
NCORES = 8
BPC = B // NCORES          # batches per core
PAIRS = C * C              # 4096
NCHUNK = PAIRS // 512      # 8 stage-B chunks of 512 pairs
TEMPERATURE = 1.0
GUMBEL_KEY = 42

_cache = {}


def _build_nc():
    import concourse.bacc as bacc
    import concourse.tile as tile
    from concourse import mybir

    fp32 = mybir.dt.float32
    bf16 = mybir.dt.bfloat16
    nc = bacc.Bacc("TRN2", target_bir_lowering=False, debug=False,
                   num_devices=NCORES)

    xa_s = nc.dram_tensor("xa_s", (BPC * C, F), bf16, kind="ExternalInput").ap()
    w1cat = nc.dram_tensor("w1cat", (F, 2 * H), bf16, kind="ExternalInput").ap()
    b1col = nc.dram_tensor("b1col", (H, 1), fp32, kind="ExternalInput").ap()
    w2d = nc.dram_tensor("w2d", (H, 1), bf16, kind="ExternalInput").ap()
    sel = nc.dram_tensor("sel", (2 * C, PAIRS), bf16, kind="ExternalInput").ap()
    gdt = nc.dram_tensor("gdt", (128, BPC * PAIRS // 128), fp32,
                         kind="ExternalInput").ap()
    ident = nc.dram_tensor("ident", (128, 128), fp32, kind="ExternalInput").ap()
    identb = nc.dram_tensor("identb", (128, 128), bf16,
                            kind="ExternalInput").ap()
    out_s = nc.dram_tensor("out_s", (BPC, PAIRS), fp32, kind="ExternalOutput").ap()

    Relu = mybir.ActivationFunctionType.Relu
    Sigmoid = mybir.ActivationFunctionType.Sigmoid
    TPC = PAIRS // 128     # 32 z-columns per batch

    with tile.TileContext(nc) as tc:
        with (
            tc.tile_pool(name="singles", bufs=1) as singles,
            tc.tile_pool(name="s_pool", bufs=BPC) as s_pool,
            tc.tile_pool(name="r1_pool", bufs=10) as r1_pool,
            tc.tile_pool(name="zg_pool", bufs=3) as zg_pool,
            tc.tile_pool(name="o_pool", bufs=3) as o_pool,
            tc.tile_pool(name="ps_misc", bufs=1, space="PSUM") as ps_misc,
            tc.tile_pool(name="ps_r", bufs=5, space="PSUM") as ps_r,
            tc.tile_pool(name="ps_z", bufs=2, space="PSUM") as ps_z,
        ):
            # ---- inputs; spread across DMA queues, critical first ----
            # xa (bf16, halves the transfer) alone on the sync queue
            xa_sb = singles.tile([128, BPC * C // 128, F], bf16)
            nc.sync.dma_start(
                out=xa_sb,
                in_=xa_s.rearrange("(ch p) f -> p ch f", p=128))
            # bf16 identity first on the scalar queue (gates the transposes)
            idb_sb = singles.tile([128, 128], bf16)
            nc.scalar.dma_start(out=idb_sb, in_=identb)
            w1_sb = singles.tile([F, 2 * H], bf16)
            nc.scalar.dma_start(out=w1_sb, in_=w1cat)
            b1_sb = singles.tile([H, 1], fp32)
            nc.scalar.dma_start(out=b1_sb, in_=b1col)
            w2_sb = singles.tile([H, 1], bf16)
            nc.scalar.dma_start(out=w2_sb, in_=w2d)
            # fp32 identity (postproc transposes) + gumbel diffs, needed later
            id_sb = singles.tile([128, 128], fp32)
            nc.scalar.dma_start(out=id_sb, in_=ident)
            gd_all = singles.tile([128, BPC * TPC], fp32)
            nc.scalar.dma_start(out=gd_all, in_=gdt)

            # sel streams on the gpsimd DMA queue, chunked so stage B can
            # start once its first chunks land
            sel_sb = singles.tile([128, PAIRS], bf16)
            for k in range(0, NCHUNK, 2):
                nc.gpsimd.dma_start(out=sel_sb[:, k * 512:(k + 2) * 512],
                                    in_=sel[:, k * 512:(k + 2) * 512])

            # ---- transpose xa: xaT[f, b*64+c], bf16 ----
            xaT = singles.tile([F, BPC * C], bf16)
            for ch in range(BPC * C // 128):
                xt_ps = ps_misc.tile([F, 128], bf16, tag="ps_misc")
                nc.tensor.transpose(xt_ps, xa_sb[:, ch, :], idb_sb)
                nc.scalar.copy(out=xaT[:, ch * 128:(ch + 1) * 128], in_=xt_ps)

            # ---- stage A for all batches upfront (tiny matmuls) ----
            s_sbs = []
            for b in range(BPC):
                ss_full = ps_z.tile([128, H], fp32, tag="ps_z")
                ss_ps = ss_full[:, 0:H]
                xaT_b = xaT[:, b * C:(b + 1) * C]
                nc.tensor.matmul(ss_ps[0:C, :], lhsT=xaT_b,
                                 rhs=w1_sb[:, 0:H], start=True, stop=True)
                nc.tensor.matmul(ss_ps[C:2 * C, :], lhsT=xaT_b,
                                 rhs=w1_sb[:, H:2 * H], start=True, stop=True,
                                 tile_position=(0, 64))
                s_sb = s_pool.tile([128, H], bf16, tag="s_sb")
                nc.vector.tensor_copy(out=s_sb, in_=ss_ps)
                s_sbs.append(s_sb)

            # ---- main pipeline: B -> relu -> D, then per-batch postproc ----
            for b in range(BPC):
                z_ps = ps_z.tile([128, TPC], fp32, tag="ps_z")
                for kk in range(0, NCHUNK, 4):
                    # four B matmuls back-to-back share the loaded S weights
                    r_pss = []
                    r1_sbs = []
                    for k in range(kk, kk + 4):
                        r_ps = ps_r.tile([128, 512], fp32, tag="ps_r")
                        nc.tensor.matmul(r_ps, lhsT=s_sbs[b],
                                         rhs=sel_sb[:, k * 512:(k + 1) * 512],
                                         start=True, stop=True)
                        r_pss.append(r_ps)
                    for k in range(kk, kk + 4):
                        r1_sb = r1_pool.tile([128, 512], bf16, tag="r1_sb")
                        if k % 2 == 0:
                            nc.scalar.activation(out=r1_sb, in_=r_pss[k - kk],
                                                 func=Relu, bias=b1_sb,
                                                 scale=1.0)
                        else:
                            nc.vector.tensor_scalar(
                                out=r1_sb, in0=r_pss[k - kk], scalar1=b1_sb,
                                scalar2=0.0, op0=mybir.AluOpType.add,
                                op1=mybir.AluOpType.max)
                        r1_sbs.append(r1_sb)
                    for k in range(kk, kk + 4):
                        for t4 in range(4):
                            t = 4 * k + t4
                            nc.tensor.matmul(
                                z_ps[:, t:t + 1],
                                lhsT=r1_sbs[k - kk][:, t4 * 128:(t4 + 1) * 128],
                                rhs=w2_sb, start=True, stop=True)

                # ---- postproc: sigmoid(Z + gd), transposed out ----
                zg_sb = zg_pool.tile([128, TPC], fp32, tag="zg_sb")
                nc.vector.tensor_add(out=zg_sb, in0=z_ps,
                                     in1=gd_all[:, b * TPC:(b + 1) * TPC])
                zt_ps = ps_misc.tile([TPC, 128], fp32, tag="ps_misc")
                nc.tensor.transpose(zt_ps, zg_sb, id_sb)
                o_sb = o_pool.tile([TPC, 128], fp32, tag="o_sb")
                nc.scalar.activation(out=o_sb, in_=zt_ps, func=Sigmoid)
                nc.sync.dma_start(
                    out=out_s[b].rearrange("(t p) -> t p", p=128), in_=o_sb)

    nc.compile()
    return nc


def _host_constants(W1, b1, W2, b2):
    import ml_dtypes
    w1cat = np.concatenate([W1[:F], W1[F:]], axis=1).astype(np.float32)
    w1cat = w1cat.astype(ml_dtypes.bfloat16)
    b1col = np.ascontiguousarray(b1.astype(np.float32).reshape(H, 1))
    w2d = np.ascontiguousarray(
        ((W2[:, 1] - W2[:, 0]) / TEMPERATURE).astype(np.float32).reshape(H, 1)
    ).astype(ml_dtypes.bfloat16)
    p = np.arange(PAIRS)
    i, j = p // C, p % C
    sel = np.zeros((2 * C, PAIRS), np.float32)
    sel[i, p] += 1.0
    sel[C + j, p] += 1.0
    sel = sel.astype(ml_dtypes.bfloat16)  # 0/1 exact in bf16
    ident = np.eye(128, dtype=np.float32)
    identb = ident.astype(ml_dtypes.bfloat16)
    b2d = float(b2[1] - b2[0])
    return w1cat, b1col, w2d, sel, ident, identb, b2d


def _gumbel_diff(b2d):
    # Must match the reference bit-for-bit: same jax call on the ambient
    # default backend (the grader runs the reference in this same env).
    import jax
    import jax.numpy as jnp
    g = jax.random.gumbel(jax.random.key(GUMBEL_KEY), (B, C, C, 2), jnp.float32)
    g = np.asarray(g)
    gd = (g[..., 1] - g[..., 0] + b2d) / TEMPERATURE   # [B, C, C]
    # device layout per batch: Z[p, t] covers pair = t*128 + p
    gdt = gd.reshape(B, PAIRS // 128, 128).transpose(0, 2, 1)  # [B, 128, 32]
    # per core: [128, BPC*32] with batch-major columns
    gdt = gdt.reshape(NCORES, BPC, 128, PAIRS // 128).transpose(0, 2, 1, 3)
    return np.ascontiguousarray(gdt.astype(np.float32)).reshape(
        NCORES, 128, BPC * (PAIRS // 128))


def kernel(xa, W1, b1, W2, b2):
    from concourse.bass_utils import run_bass_kernel_spmd

    if "nc" not in _cache:
        _cache["nc"] = _build_nc()
    nc = _cache["nc"]

    w1cat, b1col, w2d, sel, ident, identb, b2d = _host_constants(W1, b1, W2, b2)
    if _cache.get("gdt_b2d") != b2d:
        _cache["gdt"] = _gumbel_diff(b2d)
        _cache["gdt_b2d"] = b2d
    gdt = _cache["gdt"]
    import ml_dtypes
    xa_bf = np.ascontiguousarray(xa.astype(np.float32)).astype(ml_dtypes.bfloat16)

    in_maps = []
    for c in range(NCORES):
        in_maps.append({
            "xa_s": xa_bf[c * BPC:(c + 1) * BPC].reshape(BPC * C, F),
            "w1cat": w1cat, "b1col": b1col, "w2d": w2d, "sel": sel,
            "gdt": gdt[c], "ident": ident,
            "identb": identb,
        })
    res = run_bass_kernel_spmd(nc, in_maps, core_ids=list(range(NCORES)))
    _cache["last_res"] = res
    out = np.concatenate([r["out_s"] for r in res.results], axis=0)
    return out.reshape(B, C, C)
